# revision 27
# baseline (speedup 1.0000x reference)
"""CondInst fused kernel for 8 Trainium2 NeuronCores.

The reference output depends only on batch element 0 of cnn_feature:
  - params are gathered from ctrl[0] at detection centers
  - feats is a broadcast of mask_feats[0]
so the tower/controller work for batches 1..3 is dead code, and the
controller conv is only needed at the 100 detection positions.

Strategy (embarrassingly parallel, no collectives):
  - Spatially shard batch-0 across the 8 cores: 20 output rows each,
    with a 4-row halo on the input so the 4 chained 3x3 convs need no
    inter-core exchange.  Image-boundary SAME-padding is enforced by
    per-core BN scale/shift vectors that are zeroed for out-of-image
    rows (relu(x*0+0) == 0).
  - The controller conv at the 100 detection points is a tiny matmul on
    host-gathered 3x3 patches (contract dim 1152), computed on-device.
  - The dynamic mask head runs on every core for all 100 instances over
    that core's 3200 pixels:
      layer0: stacked matmul, lhsT [10, 800] shared rhs (rel-coords are
              folded into per-instance biases; the grid term is shared)
      layer1: block-diagonal matmuls, 16 instances per 128x128 tile
      layer2: per-partition scalar multiply + block-ones matmul

Layout trick: the controller weight columns are host-permuted so every
on-device rearrangement of the dynamic params is a plain contiguous DMA:
  cols   0:80   w0 stored c'*8+o, c' ordered (feats 0..8, rel-x, rel-y)
  cols  80:144  w1 stored o*8+o'  (per-instance transposed)
  cols 144:152 w2, 152:160 b0, 160:168 b1, 168 b2 (unchanged)

Compute dtype: KERNEL_DT env = "bf16" (default, full-rate matmuls,
rel err ~1e-2) or "fp32" (native fp32 matmuls, 4 passes, slower).
"""

import os
import numpy as np

B, CIN, H, W = 4, 128, 160, 160
K = 100
CH = 8
OUT = 8
STRIDE = 4
EPS = 1e-5
NCORES = 8

ROWS = H // NCORES          # 20 output rows per core
F = W + 2                   # padded row width 162
HALO = 4
RIN = ROWS + 2 * HALO       # 28 input rows per core
P3 = ROWS * F               # 3240 padded pixels per core
NCHUNK = 486                # mask-head / proj free-dim chunk (3 rows)
CONTRACT = CIN * 9          # 1152
GROUPS = [(g * 16, min(16, K - g * 16)) for g in range((K + 15) // 16)]

_CACHE = {}


def _mode():
    return os.environ.get("KERNEL_DT", "bf16")


def _param_perm():
    """new param index -> original param index (169,)

    w0's h-channel axis c is reordered to match the hbase row layout
    (feats rows 0..8, then -gx, -gy rows 8..10): c' = c-2 for feats,
    c'=8 for rel-x (orig c=0), c'=9 for rel-y (orig c=1).
    """
    perm = np.zeros(169, np.int64)
    corder = [2, 3, 4, 5, 6, 7, 8, 9, 0, 1]
    for cp, c in enumerate(corder):
        for o in range(8):
            perm[cp * 8 + o] = o * 10 + c         # w0
    for o in range(8):
        for o2 in range(8):
            perm[80 + o * 8 + o2] = 80 + o2 * 8 + o   # w1 transposed per-instance
    perm[144:169] = np.arange(144, 169)
    return perm


def _host_prep(inputs):
    """Build the 8 per-core input maps (pure numpy indexing + packing)."""
    import ml_dtypes
    cdt_np = np.float32 if _mode() == "fp32" else ml_dtypes.bfloat16

    cnn_feature = np.asarray(inputs["cnn_feature"], np.float32)
    tower_w = np.asarray(inputs["tower_w"], np.float32)
    bn_gamma = np.asarray(inputs["bn_gamma"], np.float32)
    bn_beta = np.asarray(inputs["bn_beta"], np.float32)
    bn_mean = np.asarray(inputs["bn_mean"], np.float32)
    bn_var = np.asarray(inputs["bn_var"], np.float32)
    proj_w = np.asarray(inputs["proj_w"], np.float32)
    proj_b = np.asarray(inputs["proj_b"], np.float32)
    ctrl_w = np.asarray(inputs["ctrl_w"], np.float32)
    ctrl_b = np.asarray(inputs["ctrl_b"], np.float32)
    detection = np.asarray(inputs["detection"])

    x0 = cnn_feature[0]                                   # [128, 160, 160]

    # tower weights as lhsT per tap: twT[i*9+ky*3+kx] = W[i,:,:,ky,kx].T
    twT = np.ascontiguousarray(
        tower_w.transpose(0, 3, 4, 2, 1).reshape(36, 128, 128)).astype(cdt_np)

    # BN scale/shift
    inv = bn_gamma / np.sqrt(bn_var + EPS)                # [4, 128]
    shift = bn_beta - bn_mean * inv                       # [4, 128]

    # controller weights, column-permuted, +bias row, padded to 1280 contract
    perm = _param_perm()
    cw_flat = ctrl_w.reshape(169, CONTRACT)
    cwT = np.zeros((1280, 169), np.float32)
    cwT[:CONTRACT, :] = cw_flat[perm].T
    cwT[CONTRACT, :] = ctrl_b[perm]

    # patches at detection centers, transposed, +ones row; fused with cwT
    # into one tensor so each 128-contract chunk is a single DMA (the fp32
    # matmul codegen allows only one sync wait on its weight-load slot)
    xs = detection[:, 0].astype(np.int64)
    ys = detection[:, 1].astype(np.int64)
    xpad2 = np.pad(x0, ((0, 0), (1, 1), (1, 1)))
    pcw = np.zeros((1280, K + 169), np.float32)
    for k in range(K):
        pcw[:CONTRACT, k] = xpad2[:, ys[k]:ys[k] + 3, xs[k]:xs[k] + 3].ravel()
    pcw[CONTRACT, :K] = 1.0
    pcw[:, K:] = cwT

    # detection centers replicated 8x along partitions, per 16-instance group:
    # detfan[kl*8+o, g] = 4*x_{16g+kl} (cols 0..7), 4*y (cols 7..14)
    det4 = detection.astype(np.float32) * STRIDE
    detfan = np.zeros((128, 14), np.float32)
    for g, (k0, gsz) in enumerate(GROUPS):
        for kl in range(gsz):
            detfan[kl * 8:kl * 8 + 8, g] = det4[k0 + kl, 0]
            detfan[kl * 8:kl * 8 + 8, 7 + g] = det4[k0 + kl, 1]

    onesbd = np.zeros((128, 16), np.float32)
    for kl in range(16):
        onesbd[kl * 8:kl * 8 + 8, kl] = 1.0

    projT = np.ascontiguousarray(proj_w.T).astype(cdt_np)  # [128, 8]
    projb = proj_b.reshape(8, 1).astype(np.float32)

    # per-core padded input slices
    xpad_rows = np.zeros((128, H + 2 * HALO, F), np.float32)
    xpad_rows[:, HALO:HALO + H, 1:161] = x0
    xpad_rows = xpad_rows.astype(cdt_np)

    shared = dict(twT=twT, pcw=pcw.astype(cdt_np), detfan=detfan,
                  onesbd=onesbd.astype(cdt_np), projT=projT, projb=projb)

    in_maps = []
    for c in range(NCORES):
        xin = np.ascontiguousarray(xpad_rows[:, ROWS * c:ROWS * c + RIN, :])

        # bnv[ch, i, region, 0/1] = inv/shift; zeroed for out-of-image regions
        bnv = np.zeros((128, 4, 3, 2), np.float32)
        for i in range(4):
            bnv[:, i, 1, 0] = inv[i]
            bnv[:, i, 1, 1] = shift[i]
            if c != 0:
                bnv[:, i, 0, 0] = inv[i]
                bnv[:, i, 0, 1] = shift[i]
            if c != NCORES - 1:
                bnv[:, i, 2, 0] = inv[i]
                bnv[:, i, 2, 1] = shift[i]

        grid = np.zeros((2, ROWS, F), np.float32)
        gxrow = -(np.arange(W, dtype=np.float32) * STRIDE + STRIDE // 2)
        gyv = -(np.arange(ROWS * c, ROWS * c + ROWS, dtype=np.float32) * STRIDE
                + STRIDE // 2)
        grid[0, :, 1:161] = gxrow[None, :]
        grid[1, :, 1:161] = gyv[:, None]

        in_maps.append(dict(shared, xin=xin,
                            bnv=bnv.reshape(128, 24),
                            grid=grid.reshape(2, ROWS * F).astype(cdt_np)))
    return in_maps


def _build_program(reps=1):
    from contextlib import ExitStack
    import concourse.tile as tile
    from concourse import bacc, mybir

    f32 = mybir.dt.float32
    cdt = f32 if _mode() == "fp32" else mybir.dt.bfloat16
    Relu = mybir.ActivationFunctionType.Relu
    Ident = mybir.ActivationFunctionType.Identity

    nc = bacc.Bacc("TRN2", target_bir_lowering=False, debug=False,
                   enable_asserts=False, detect_race_conditions=False)

    xin_d = nc.dram_tensor("xin", [128, RIN, F], cdt, kind="ExternalInput")
    twT_d = nc.dram_tensor("twT", [36, 128, 128], cdt, kind="ExternalInput")
    bnv_d = nc.dram_tensor("bnv", [128, 24], f32, kind="ExternalInput")
    grid_d = nc.dram_tensor("grid", [2, P3], cdt, kind="ExternalInput")
    pcw_d = nc.dram_tensor("pcw", [1280, K + 169], cdt, kind="ExternalInput")
    detfan_d = nc.dram_tensor("detfan", [128, 14], f32, kind="ExternalInput")
    onesbd_d = nc.dram_tensor("onesbd", [128, 16], cdt, kind="ExternalInput")
    projT_d = nc.dram_tensor("projT", [128, 8], cdt, kind="ExternalInput")
    projb_d = nc.dram_tensor("projb", [8, 1], f32, kind="ExternalInput")
    out_d = nc.dram_tensor("out", [K, ROWS, W], f32, kind="ExternalOutput")

    with tile.TileContext(nc) as tc, ExitStack() as octx:
      for rep in range(reps):
       with ExitStack() as ctx:
        const = ctx.enter_context(tc.tile_pool(name=f"const{rep}", bufs=1))
        prep = ctx.enter_context(tc.tile_pool(name=f"prep{rep}", bufs=1))

        # ---------- phase A: dynamic params P2[k, j'] = patches @ ctrl ----------
        with tc.tile_pool(name="pm", bufs=1) as pm, \
             tc.tile_pool(name="pm_ps", bufs=1, space="PSUM") as pm_ps:
            pc = []
            for i in range(10):
                t = pm.tile([128, K + 169], cdt, tag=f"pc{i}")
                nc.sync.dma_start(out=t[:], in_=pcw_d[128 * i:128 * (i + 1), :])
                pc.append(t)
            p2p = pm_ps.tile([K, 169], f32)
            for i in range(10):
                nc.tensor.matmul(p2p[:], lhsT=pc[i][:, 0:K], rhs=pc[i][:, K:],
                                 start=(i == 0), stop=(i == 9))
            p2 = prep.tile([K, 169], cdt)
            nc.vector.tensor_copy(p2[:], p2p[:])

        # ---------- phase B: head weight assembly (DMA/DVE, overlaps tower) ----
        detfan_sb = const.tile([128, 14], f32)
        nc.sync.dma_start(out=detfan_sb[:], in_=detfan_d[:])
        onesbd_sb = const.tile([128, 16], cdt)
        nc.sync.dma_start(out=onesbd_sb[:], in_=onesbd_d[:])

        # lhsT0 [10, 800]: l0[c, k*8+o] = P2[k, c*8+o]
        l0 = prep.tile([10, 8 * K], cdt)
        for c in range(10):
            nc.sync.dma_start(
                out=l0[c:c + 1, :].rearrange("p (k o) -> p k o", o=8),
                in_=p2[:, c * 8:(c + 1) * 8])

        # fan-out columns: [kl*8+o, g] layout, one column per instance group
        b0fan = prep.tile([128, 7], f32)
        w0xfan = prep.tile([128, 7], f32)
        w0yfan = prep.tile([128, 7], f32)
        b1fan = prep.tile([128, 7], f32)
        w2fan = prep.tile([128, 7], f32)
        b2fan = prep.tile([16, 7], f32)
        for fan in (b0fan, w0xfan, w0yfan, b1fan, w2fan, b2fan):
            nc.gpsimd.memset(fan[:], 0.0)
        bd1 = []
        for g, (k0, gsz) in enumerate(GROUPS):
            gp = gsz * 8
            bd = prep.tile([gp, gp], cdt, tag=f"bd{g}")
            nc.gpsimd.memset(bd[:], 0.0)
            for kl in range(gsz):
                k = k0 + kl
                nc.sync.dma_start(out=bd[kl * 8:kl * 8 + 8, kl * 8:kl * 8 + 8],
                                  in_=p2[k:k + 1, 80:144])
            bd1.append(bd)
            for fan, c0 in ((b0fan, 152), (w0xfan, 64), (w0yfan, 72),
                            (b1fan, 160), (w2fan, 144)):
                nc.gpsimd.dma_start(out=fan[0:gp, g:g + 1],
                                    in_=p2[k0:k0 + gsz, c0:c0 + 8])
            nc.gpsimd.dma_start(out=b2fan[0:gsz, g:g + 1],
                                in_=p2[k0:k0 + gsz, 168:169])

        # beta0fan = b0fan + w0xfan*4x + w0yfan*4y  (per-instance bias, layer 0)
        beta0fan = prep.tile([128, 7], f32)
        tmpf = prep.tile([128, 7], f32)
        nc.vector.tensor_mul(beta0fan[:], w0xfan[:], detfan_sb[:, 0:7])
        nc.vector.tensor_mul(tmpf[:], w0yfan[:], detfan_sb[:, 7:14])
        nc.vector.tensor_add(beta0fan[:], beta0fan[:], tmpf[:])
        nc.vector.tensor_add(beta0fan[:], beta0fan[:], b0fan[:])

        # ---------- phase C: conv tower on this core's row slice ----------
        tw_sb = []
        for t in range(36):
            wt = const.tile([128, 128], cdt, tag=f"tw{t}")
            nc.sync.dma_start(out=wt[:], in_=twT_d[t])
            tw_sb.append(wt)
        bnv_sb = const.tile([128, 24], f32)
        nc.sync.dma_start(out=bnv_sb[:], in_=bnv_d[:])

        hbase = const.tile([10, P3], cdt)
        nc.sync.dma_start(out=hbase[8:10, :], in_=grid_d[:])

        with tc.tile_pool(name="conv", bufs=1) as convp, \
             tc.tile_pool(name="conv_ps", bufs=4, space="PSUM") as conv_ps:
            xbuf = convp.tile([128, RIN * F + 2], cdt, tag="xbuf")
            nc.gpsimd.memset(xbuf[:, 0:1], 0.0)
            nc.gpsimd.memset(xbuf[:, 1 + RIN * F:], 0.0)
            nc.sync.dma_start(out=xbuf[:, 1:1 + RIN * F], in_=xin_d[:])

            cur = xbuf
            rcur = RIN
            for i in range(4):
                rout = rcur - 2
                obuf = convp.tile([128, rout * F + 2], cdt, tag=f"c{i}")
                nc.gpsimd.memset(obuf[:], 0.0)
                obuf3 = obuf[:, 1:1 + rout * F].rearrange("p (r c) -> p r c", c=F)

                T = 3 - i  # out-of-image candidate rows at top/bottom
                bounds = sorted({0, T, rout - T, rout})
                for r0 in range(0, rout, 3):
                    nr = min(3, rout - r0)
                    ps = conv_ps.tile([128, nr * F], f32, tag="cps")
                    for t, (ky, kx) in enumerate(
                            (ky, kx) for ky in range(3) for kx in range(3)):
                        off = 1 + (r0 + ky) * F + kx - 1
                        nc.tensor.matmul(
                            ps[:], lhsT=tw_sb[i * 9 + t][:],
                            rhs=cur[:, off:off + nr * F],
                            start=(t == 0), stop=(t == 8))
                    ps3 = ps[:].rearrange("p (r c) -> p r c", c=F)
                    # split chunk rows by (top|mid|bot) BN regions
                    for rs, re in zip(bounds[:-1], bounds[1:]):
                        a, b = max(rs, r0), min(re, r0 + nr)
                        if a >= b:
                            continue
                        reg = 0 if b <= T else (2 if a >= rout - T else 1)
                        sidx = (i * 3 + reg) * 2
                        nc.scalar.activation(
                            out=obuf3[:, a:b, 1:161],
                            in_=ps3[:, a - r0:b - r0, 1:161],
                            func=Relu,
                            scale=bnv_sb[:, sidx:sidx + 1],
                            bias=bnv_sb[:, sidx + 1:sidx + 2])
                cur = obuf
                rcur = rout

            # ---------- proj: mask_feats -> hbase rows 0..8 ----------
            projT_sb = const.tile([128, 8], cdt)
            nc.sync.dma_start(out=projT_sb[:], in_=projT_d[:])
            projb_sb = const.tile([8, 1], f32)
            nc.sync.dma_start(out=projb_sb[:], in_=projb_d[:])

            c4 = cur[:, 1:1 + P3]
            with tc.tile_pool(name="proj_ps", bufs=2, space="PSUM") as proj_ps:
                for n0 in range(0, P3, NCHUNK):
                    nn = min(NCHUNK, P3 - n0)
                    pp = proj_ps.tile([8, nn], f32, tag="pps")
                    nc.tensor.matmul(pp[:], lhsT=projT_sb[:],
                                     rhs=c4[:, n0:n0 + nn],
                                     start=True, stop=True)
                    nc.scalar.activation(out=hbase[0:8, n0:n0 + nn], in_=pp[:],
                                         func=Ident, bias=projb_sb[:, 0:1])

        # ---------- phase D: dynamic mask head ----------
        with tc.tile_pool(name="head", bufs=3) as headp, \
             tc.tile_pool(name="outp", bufs=2) as outp, \
             tc.tile_pool(name="head_ps", bufs=2, space="PSUM") as head_ps:
            for g, (k0, gsz) in enumerate(GROUPS):
                gp = gsz * 8
                outg = outp.tile([gsz, P3], f32, tag="outg")
                for n0 in range(0, P3, NCHUNK):
                    nn = min(NCHUNK, P3 - n0)
                    ps0 = hps0.tile([gp, nn], f32, tag="ps0")
                    nc.tensor.matmul(ps0[:],
                                     lhsT=l0[:, 8 * k0:8 * k0 + gp],
                                     rhs=hbase[:, n0:n0 + nn],
                                     start=True, stop=True)
                    h1c = headp.tile([gp, nn], cdt, tag="h1c")
                    nc.scalar.activation(out=h1c[:], in_=ps0[:], func=Relu,
                                         bias=beta0fan[0:gp, g:g + 1])
                    ps1 = hps1.tile([gp, nn], f32, tag="ps1")
                    nc.tensor.matmul(ps1[:], lhsT=bd1[g][:], rhs=h1c[:],
                                     start=True, stop=True)
                    h2c = headp.tile([gp, nn], cdt, tag="h2c")
                    nc.scalar.activation(out=h2c[:], in_=ps1[:], func=Relu,
                                         bias=b1fan[0:gp, g:g + 1])
                    h2w = headp.tile([gp, nn], cdt, tag="h2w")
                    nc.vector.tensor_scalar_mul(h2w[:], h2c[:],
                                                w2fan[0:gp, g:g + 1])
                    ps2 = hps2.tile([gsz, nn], f32, tag="ps2")
                    nc.tensor.matmul(ps2[:],
                                     lhsT=onesbd_sb[0:gp, 0:gsz], rhs=h2w[:],
                                     start=True, stop=True)
                    nc.scalar.activation(out=outg[:, n0:n0 + nn], in_=ps2[:],
                                         func=Ident,
                                         bias=b2fan[0:gsz, g:g + 1])
                outg3 = outg[:].rearrange("p (r c) -> p r c", c=F)
                nc.sync.dma_start(out=out_d[k0:k0 + gsz, 0:10, :],
                                  in_=outg3[:, 0:10, 1:161])
                nc.sync.dma_start(out=out_d[k0:k0 + gsz, 10:20, :],
                                  in_=outg3[:, 10:20, 1:161])
    nc.compile()
    return nc


def _get_program(reps=1):
    key = (_mode(), reps)
    if key not in _CACHE:
        _CACHE[key] = _build_program(reps)
    return _CACHE[key]


def _run(in_maps, trace=False, reps=1, **kwargs):
    from concourse.bass_utils import run_bass_kernel_spmd
    nc = _get_program(reps)
    return run_bass_kernel_spmd(nc, in_maps, core_ids=list(range(NCORES)),
                                trace=trace, **kwargs)


def kernel(**inputs) -> np.ndarray:
    in_maps = _host_prep(inputs)
    res = _run(in_maps)
    out = np.concatenate([res.results[c]["out"] for c in range(NCORES)], axis=1)
    return out.astype(np.float32)


# revision 29
# speedup vs baseline: 1.1125x; 1.1125x over previous
"""CondInst fused kernel for 8 Trainium2 NeuronCores.

The reference output depends only on batch element 0 of cnn_feature:
  - params are gathered from ctrl[0] at detection centers
  - feats is a broadcast of mask_feats[0]
so the tower/controller work for batches 1..3 is dead code, and the
controller conv is only needed at the 100 detection positions.

Strategy (embarrassingly parallel, no collectives):
  - Spatially shard batch-0 across the 8 cores: 20 output rows each,
    with a 4-row halo on the input so the 4 chained 3x3 convs need no
    inter-core exchange.  Image-boundary SAME-padding is enforced by
    per-core BN scale/shift vectors that are zeroed for out-of-image
    rows (relu(x*0+0) == 0).
  - The controller conv at the 100 detection points is a tiny matmul on
    host-gathered 3x3 patches (contract dim 1152), computed on-device.
  - The dynamic mask head runs on every core for all 100 instances over
    that core's 3200 pixels:
      layer0: stacked matmul, lhsT [10, 800] shared rhs (rel-coords are
              folded into per-instance biases; the grid term is shared)
      layer1: block-diagonal matmuls, 16 instances per 128x128 tile
      layer2: per-partition scalar multiply + block-ones matmul

Layout trick: the controller weight columns are host-permuted so every
on-device rearrangement of the dynamic params is a plain contiguous DMA:
  cols   0:80   w0 stored c'*8+o, c' ordered (feats 0..8, rel-x, rel-y)
  cols  80:144  w1 stored o*8+o'  (per-instance transposed)
  cols 144:152 w2, 152:160 b0, 160:168 b1, 168 b2 (unchanged)

Compute dtype: KERNEL_DT env = "bf16" (default, full-rate matmuls,
rel err ~1e-2) or "fp32" (native fp32 matmuls, 4 passes, slower).
"""

import os
import numpy as np

B, CIN, H, W = 4, 128, 160, 160
K = 100
CH = 8
OUT = 8
STRIDE = 4
EPS = 1e-5
NCORES = 8

ROWS = H // NCORES          # 20 output rows per core
F = W + 2                   # padded row width 162
HALO = 4
RIN = ROWS + 2 * HALO       # 28 input rows per core
P3 = ROWS * F               # 3240 padded pixels per core
NCHUNK = 486                # mask-head / proj free-dim chunk (3 rows)
CONTRACT = CIN * 9          # 1152
GROUPS = [(g * 16, min(16, K - g * 16)) for g in range((K + 15) // 16)]

_CACHE = {}


def _mode():
    return os.environ.get("KERNEL_DT", "bf16")


def _param_perm():
    """new param index -> original param index (169,)

    w0's h-channel axis c is reordered to match the hbase row layout
    (feats rows 0..8, then -gx, -gy rows 8..10): c' = c-2 for feats,
    c'=8 for rel-x (orig c=0), c'=9 for rel-y (orig c=1).
    """
    perm = np.zeros(169, np.int64)
    corder = [2, 3, 4, 5, 6, 7, 8, 9, 0, 1]
    for cp, c in enumerate(corder):
        for o in range(8):
            perm[cp * 8 + o] = o * 10 + c         # w0
    for o in range(8):
        for o2 in range(8):
            perm[80 + o * 8 + o2] = 80 + o2 * 8 + o   # w1 transposed per-instance
    perm[144:169] = np.arange(144, 169)
    return perm


def _host_prep(inputs):
    """Build the 8 per-core input maps (pure numpy indexing + packing)."""
    import ml_dtypes
    cdt_np = np.float32 if _mode() == "fp32" else ml_dtypes.bfloat16

    cnn_feature = np.asarray(inputs["cnn_feature"], np.float32)
    tower_w = np.asarray(inputs["tower_w"], np.float32)
    bn_gamma = np.asarray(inputs["bn_gamma"], np.float32)
    bn_beta = np.asarray(inputs["bn_beta"], np.float32)
    bn_mean = np.asarray(inputs["bn_mean"], np.float32)
    bn_var = np.asarray(inputs["bn_var"], np.float32)
    proj_w = np.asarray(inputs["proj_w"], np.float32)
    proj_b = np.asarray(inputs["proj_b"], np.float32)
    ctrl_w = np.asarray(inputs["ctrl_w"], np.float32)
    ctrl_b = np.asarray(inputs["ctrl_b"], np.float32)
    detection = np.asarray(inputs["detection"])

    x0 = cnn_feature[0]                                   # [128, 160, 160]

    # tower weights as lhsT per tap: twT[i*9+ky*3+kx] = W[i,:,:,ky,kx].T
    twT = np.ascontiguousarray(
        tower_w.transpose(0, 3, 4, 2, 1).reshape(36, 128, 128)).astype(cdt_np)

    # BN scale/shift
    inv = bn_gamma / np.sqrt(bn_var + EPS)                # [4, 128]
    shift = bn_beta - bn_mean * inv                       # [4, 128]

    # controller weights, column-permuted, +bias row, padded to 1280 contract
    perm = _param_perm()
    cw_flat = ctrl_w.reshape(169, CONTRACT)
    cwT = np.zeros((1280, 169), np.float32)
    cwT[:CONTRACT, :] = cw_flat[perm].T
    cwT[CONTRACT, :] = ctrl_b[perm]

    # patches at detection centers, transposed, +ones row; fused with cwT
    # into one tensor so each 128-contract chunk is a single DMA (the fp32
    # matmul codegen allows only one sync wait on its weight-load slot)
    xs = detection[:, 0].astype(np.int64)
    ys = detection[:, 1].astype(np.int64)
    xpad2 = np.pad(x0, ((0, 0), (1, 1), (1, 1)))
    pcw = np.zeros((1280, K + 169), np.float32)
    for k in range(K):
        pcw[:CONTRACT, k] = xpad2[:, ys[k]:ys[k] + 3, xs[k]:xs[k] + 3].ravel()
    pcw[CONTRACT, :K] = 1.0
    pcw[:, K:] = cwT

    # detection centers replicated 8x along partitions, per 16-instance group:
    # detfan[kl*8+o, g] = 4*x_{16g+kl} (cols 0..7), 4*y (cols 7..14)
    det4 = detection.astype(np.float32) * STRIDE
    detfan = np.zeros((128, 14), np.float32)
    for g, (k0, gsz) in enumerate(GROUPS):
        for kl in range(gsz):
            detfan[kl * 8:kl * 8 + 8, g] = det4[k0 + kl, 0]
            detfan[kl * 8:kl * 8 + 8, 7 + g] = det4[k0 + kl, 1]

    onesbd = np.zeros((128, 16), np.float32)
    for kl in range(16):
        onesbd[kl * 8:kl * 8 + 8, kl] = 1.0

    projT = np.ascontiguousarray(proj_w.T).astype(cdt_np)  # [128, 8]
    projb = proj_b.reshape(8, 1).astype(np.float32)

    # per-core padded input slices
    xpad_rows = np.zeros((128, H + 2 * HALO, F), np.float32)
    xpad_rows[:, HALO:HALO + H, 1:161] = x0
    xpad_rows = xpad_rows.astype(cdt_np)

    shared = dict(twT=twT, pcw=pcw.astype(cdt_np), detfan=detfan,
                  onesbd=onesbd.astype(cdt_np), projT=projT, projb=projb)

    in_maps = []
    for c in range(NCORES):
        xin = np.ascontiguousarray(xpad_rows[:, ROWS * c:ROWS * c + RIN, :])

        # bnv[ch, i, region, 0/1] = inv/shift; zeroed for out-of-image regions
        bnv = np.zeros((128, 4, 3, 2), np.float32)
        for i in range(4):
            bnv[:, i, 1, 0] = inv[i]
            bnv[:, i, 1, 1] = shift[i]
            if c != 0:
                bnv[:, i, 0, 0] = inv[i]
                bnv[:, i, 0, 1] = shift[i]
            if c != NCORES - 1:
                bnv[:, i, 2, 0] = inv[i]
                bnv[:, i, 2, 1] = shift[i]

        grid = np.zeros((2, ROWS, F), np.float32)
        gxrow = -(np.arange(W, dtype=np.float32) * STRIDE + STRIDE // 2)
        gyv = -(np.arange(ROWS * c, ROWS * c + ROWS, dtype=np.float32) * STRIDE
                + STRIDE // 2)
        grid[0, :, 1:161] = gxrow[None, :]
        grid[1, :, 1:161] = gyv[:, None]

        in_maps.append(dict(shared, xin=xin,
                            bnv=bnv.reshape(128, 24),
                            grid=grid.reshape(2, ROWS * F).astype(cdt_np)))
    return in_maps


def _build_program(reps=1):
    from contextlib import ExitStack
    import concourse.tile as tile
    from concourse import bacc, mybir

    f32 = mybir.dt.float32
    cdt = f32 if _mode() == "fp32" else mybir.dt.bfloat16
    Relu = mybir.ActivationFunctionType.Relu
    Ident = mybir.ActivationFunctionType.Identity

    nc = bacc.Bacc("TRN2", target_bir_lowering=False, debug=False,
                   enable_asserts=False, detect_race_conditions=False)

    xin_d = nc.dram_tensor("xin", [128, RIN, F], cdt, kind="ExternalInput")
    twT_d = nc.dram_tensor("twT", [36, 128, 128], cdt, kind="ExternalInput")
    bnv_d = nc.dram_tensor("bnv", [128, 24], f32, kind="ExternalInput")
    grid_d = nc.dram_tensor("grid", [2, P3], cdt, kind="ExternalInput")
    pcw_d = nc.dram_tensor("pcw", [1280, K + 169], cdt, kind="ExternalInput")
    detfan_d = nc.dram_tensor("detfan", [128, 14], f32, kind="ExternalInput")
    onesbd_d = nc.dram_tensor("onesbd", [128, 16], cdt, kind="ExternalInput")
    projT_d = nc.dram_tensor("projT", [128, 8], cdt, kind="ExternalInput")
    projb_d = nc.dram_tensor("projb", [8, 1], f32, kind="ExternalInput")
    out_d = nc.dram_tensor("out", [K, ROWS, W], f32, kind="ExternalOutput")

    with tile.TileContext(nc) as tc, ExitStack() as octx:
      for rep in range(reps):
       with ExitStack() as ctx:
        const = ctx.enter_context(tc.tile_pool(name=f"const{rep}", bufs=1))
        prep = ctx.enter_context(tc.tile_pool(name=f"prep{rep}", bufs=1))

        # ---------- phase A: dynamic params P2[k, j'] = patches @ ctrl ----------
        with tc.tile_pool(name="pm", bufs=1) as pm, \
             tc.tile_pool(name="pm_ps", bufs=1, space="PSUM") as pm_ps:
            pc = []
            for i in range(10):
                t = pm.tile([128, K + 169], cdt, tag=f"pc{i}")
                nc.sync.dma_start(out=t[:], in_=pcw_d[128 * i:128 * (i + 1), :])
                pc.append(t)
            p2p = pm_ps.tile([K, 169], f32)
            for i in range(10):
                nc.tensor.matmul(p2p[:], lhsT=pc[i][:, 0:K], rhs=pc[i][:, K:],
                                 start=(i == 0), stop=(i == 9))
            p2 = prep.tile([K, 169], cdt)
            nc.vector.tensor_copy(p2[:], p2p[:])

        # ---------- phase B: head weight assembly (DMA/DVE, overlaps tower) ----
        detfan_sb = const.tile([128, 14], f32)
        nc.sync.dma_start(out=detfan_sb[:], in_=detfan_d[:])
        onesbd_sb = const.tile([128, 16], cdt)
        nc.sync.dma_start(out=onesbd_sb[:], in_=onesbd_d[:])

        # lhsT0 [10, 800]: l0[c, k*8+o] = P2[k, c*8+o]
        l0 = prep.tile([10, 8 * K], cdt)
        for c in range(10):
            nc.sync.dma_start(
                out=l0[c:c + 1, :].rearrange("p (k o) -> p k o", o=8),
                in_=p2[:, c * 8:(c + 1) * 8])

        # fan-out columns: [kl*8+o, g] layout, one column per instance group
        b0fan = prep.tile([128, 7], f32)
        w0xfan = prep.tile([128, 7], f32)
        w0yfan = prep.tile([128, 7], f32)
        b1fan = prep.tile([128, 7], f32)
        w2fan = prep.tile([128, 7], f32)
        b2fan = prep.tile([16, 7], f32)
        for fan in (b0fan, w0xfan, w0yfan, b1fan, w2fan, b2fan):
            nc.gpsimd.memset(fan[:], 0.0)
        bd1 = []
        for g, (k0, gsz) in enumerate(GROUPS):
            gp = gsz * 8
            bd = prep.tile([gp, gp], cdt, tag=f"bd{g}")
            nc.gpsimd.memset(bd[:], 0.0)
            for kl in range(gsz):
                k = k0 + kl
                nc.sync.dma_start(out=bd[kl * 8:kl * 8 + 8, kl * 8:kl * 8 + 8],
                                  in_=p2[k:k + 1, 80:144])
            bd1.append(bd)
            for fan, c0 in ((b0fan, 152), (w0xfan, 64), (w0yfan, 72),
                            (b1fan, 160), (w2fan, 144)):
                nc.gpsimd.dma_start(out=fan[0:gp, g:g + 1],
                                    in_=p2[k0:k0 + gsz, c0:c0 + 8])
            nc.gpsimd.dma_start(out=b2fan[0:gsz, g:g + 1],
                                in_=p2[k0:k0 + gsz, 168:169])

        # beta0fan = b0fan + w0xfan*4x + w0yfan*4y  (per-instance bias, layer 0)
        beta0fan = prep.tile([128, 7], f32)
        tmpf = prep.tile([128, 7], f32)
        nc.vector.tensor_mul(beta0fan[:], w0xfan[:], detfan_sb[:, 0:7])
        nc.vector.tensor_mul(tmpf[:], w0yfan[:], detfan_sb[:, 7:14])
        nc.vector.tensor_add(beta0fan[:], beta0fan[:], tmpf[:])
        nc.vector.tensor_add(beta0fan[:], beta0fan[:], b0fan[:])

        # ---------- phase C: conv tower on this core's row slice ----------
        tw_sb = []
        for t in range(36):
            wt = const.tile([128, 128], cdt, tag=f"tw{t}")
            nc.sync.dma_start(out=wt[:], in_=twT_d[t])
            tw_sb.append(wt)
        bnv_sb = const.tile([128, 24], f32)
        nc.sync.dma_start(out=bnv_sb[:], in_=bnv_d[:])

        hbase = const.tile([10, P3], cdt)
        nc.sync.dma_start(out=hbase[8:10, :], in_=grid_d[:])

        with tc.tile_pool(name="conv", bufs=1) as convp, \
             tc.tile_pool(name="conv_ps", bufs=4, space="PSUM") as conv_ps:
            xbuf = convp.tile([128, RIN * F + 2], cdt, tag="xbuf")
            nc.gpsimd.memset(xbuf[:, 0:1], 0.0)
            nc.gpsimd.memset(xbuf[:, 1 + RIN * F:], 0.0)
            nc.sync.dma_start(out=xbuf[:, 1:1 + RIN * F], in_=xin_d[:])

            cur = xbuf
            rcur = RIN
            for i in range(4):
                rout = rcur - 2
                obuf = convp.tile([128, rout * F + 2], cdt, tag=f"c{i}")
                nc.gpsimd.memset(obuf[:], 0.0)
                obuf3 = obuf[:, 1:1 + rout * F].rearrange("p (r c) -> p r c", c=F)

                T = 3 - i  # out-of-image candidate rows at top/bottom
                bounds = sorted({0, T, rout - T, rout})
                for r0 in range(0, rout, 3):
                    nr = min(3, rout - r0)
                    ps = conv_ps.tile([128, nr * F], f32, tag="cps")
                    for t, (ky, kx) in enumerate(
                            (ky, kx) for ky in range(3) for kx in range(3)):
                        off = 1 + (r0 + ky) * F + kx - 1
                        nc.tensor.matmul(
                            ps[:], lhsT=tw_sb[i * 9 + t][:],
                            rhs=cur[:, off:off + nr * F],
                            start=(t == 0), stop=(t == 8))
                    ps3 = ps[:].rearrange("p (r c) -> p r c", c=F)
                    # split chunk rows by (top|mid|bot) BN regions
                    for rs, re in zip(bounds[:-1], bounds[1:]):
                        a, b = max(rs, r0), min(re, r0 + nr)
                        if a >= b:
                            continue
                        reg = 0 if b <= T else (2 if a >= rout - T else 1)
                        sidx = (i * 3 + reg) * 2
                        nc.scalar.activation(
                            out=obuf3[:, a:b, 1:161],
                            in_=ps3[:, a - r0:b - r0, 1:161],
                            func=Relu,
                            scale=bnv_sb[:, sidx:sidx + 1],
                            bias=bnv_sb[:, sidx + 1:sidx + 2])
                cur = obuf
                rcur = rout

            # ---------- proj: mask_feats -> hbase rows 0..8 ----------
            projT_sb = const.tile([128, 8], cdt)
            nc.sync.dma_start(out=projT_sb[:], in_=projT_d[:])
            projb_sb = const.tile([8, 1], f32)
            nc.sync.dma_start(out=projb_sb[:], in_=projb_d[:])

            c4 = cur[:, 1:1 + P3]
            with tc.tile_pool(name="proj_ps", bufs=2, space="PSUM") as proj_ps:
                for n0 in range(0, P3, NCHUNK):
                    nn = min(NCHUNK, P3 - n0)
                    pp = proj_ps.tile([8, nn], f32, tag="pps")
                    nc.tensor.matmul(pp[:], lhsT=projT_sb[:],
                                     rhs=c4[:, n0:n0 + nn],
                                     start=True, stop=True)
                    nc.scalar.activation(out=hbase[0:8, n0:n0 + nn], in_=pp[:],
                                         func=Ident, bias=projb_sb[:, 0:1])

        # ---------- phase D: dynamic mask head ----------
        with tc.tile_pool(name="head", bufs=3) as headp, \
             tc.tile_pool(name="outp", bufs=2) as outp, \
             tc.tile_pool(name="head_ps", bufs=2, space="PSUM") as head_ps:
            for g, (k0, gsz) in enumerate(GROUPS):
                gp = gsz * 8
                outg = outp.tile([gsz, P3], f32, tag="outg")
                for n0 in range(0, P3, NCHUNK):
                    nn = min(NCHUNK, P3 - n0)
                    ps0 = hps0.tile([gp, nn], f32, tag="ps0")
                    nc.tensor.matmul(ps0[:],
                                     lhsT=l0[:, 8 * k0:8 * k0 + gp],
                                     rhs=hbase[:, n0:n0 + nn],
                                     start=True, stop=True)
                    h1c = headp.tile([gp, nn], cdt, tag="h1c")
                    nc.scalar.activation(out=h1c[:], in_=ps0[:], func=Relu,
                                         bias=beta0fan[0:gp, g:g + 1])
                    ps1 = hps1.tile([gp, nn], f32, tag="ps1")
                    nc.tensor.matmul(ps1[:], lhsT=bd1[g][:], rhs=h1c[:],
                                     start=True, stop=True)
                    h2c = headp.tile([gp, nn], cdt, tag="h2c")
                    nc.scalar.activation(out=h2c[:], in_=ps1[:], func=Relu,
                                         bias=b1fan[0:gp, g:g + 1])
                    h2w = headp.tile([gp, nn], cdt, tag="h2w")
                    nc.vector.tensor_scalar_mul(h2w[:], h2c[:],
                                                w2fan[0:gp, g:g + 1])
                    ps2 = hps2.tile([gsz, nn], f32, tag="ps2")
                    nc.tensor.matmul(ps2[:],
                                     lhsT=onesbd_sb[0:gp, 0:gsz], rhs=h2w[:],
                                     start=True, stop=True)
                    nc.scalar.activation(out=outg[:, n0:n0 + nn], in_=ps2[:],
                                         func=Ident,
                                         bias=b2fan[0:gsz, g:g + 1])
                outg3 = outg[:].rearrange("p (r c) -> p r c", c=F)
                nc.sync.dma_start(out=out_d[k0:k0 + gsz, 0:10, :],
                                  in_=outg3[:, 0:10, 1:161])
                nc.sync.dma_start(out=out_d[k0:k0 + gsz, 10:20, :],
                                  in_=outg3[:, 10:20, 1:161])
    nc.compile()
    return nc


def _get_program(reps=1):
    key = (_mode(), reps)
    if key not in _CACHE:
        _CACHE[key] = _build_program(reps)
    return _CACHE[key]


def _run(in_maps, trace=False, reps=1, **kwargs):
    from concourse.bass_utils import run_bass_kernel_spmd
    nc = _get_program(reps)
    return run_bass_kernel_spmd(nc, in_maps, core_ids=list(range(NCORES)),
                                trace=trace, **kwargs)


def kernel(**inputs) -> np.ndarray:
    in_maps = _host_prep(inputs)
    res = _run(in_maps)
    out = np.concatenate([res.results[c]["out"] for c in range(NCORES)], axis=1)
    return out.astype(np.float32)


# revision 31
# speedup vs baseline: 1.1151x; 1.0024x over previous
"""CondInst fused kernel for 8 Trainium2 NeuronCores.

The reference output depends only on batch element 0 of cnn_feature:
  - params are gathered from ctrl[0] at detection centers
  - feats is a broadcast of mask_feats[0]
so the tower/controller work for batches 1..3 is dead code, and the
controller conv is only needed at the 100 detection positions.

Strategy (embarrassingly parallel, no collectives):
  - Spatially shard batch-0 across the 8 cores: 20 output rows each,
    with a 4-row halo on the input so the 4 chained 3x3 convs need no
    inter-core exchange.  Image-boundary SAME-padding is enforced by
    per-core BN scale/shift vectors that are zeroed for out-of-image
    rows (relu(x*0+0) == 0).
  - The controller conv at the 100 detection points is a tiny matmul on
    host-gathered 3x3 patches (contract dim 1152), computed on-device.
  - The dynamic mask head runs on every core for all 100 instances over
    that core's 3200 pixels:
      layer0: stacked matmul, lhsT [10, 800] shared rhs (rel-coords are
              folded into per-instance biases; the grid term is shared)
      layer1: block-diagonal matmuls, 16 instances per 128x128 tile
      layer2: per-partition scalar multiply + block-ones matmul

Layout trick: the controller weight columns are host-permuted so every
on-device rearrangement of the dynamic params is a plain contiguous DMA:
  cols   0:80   w0 stored c'*8+o, c' ordered (feats 0..8, rel-x, rel-y)
  cols  80:144  w1 stored o*8+o'  (per-instance transposed)
  cols 144:152 w2, 152:160 b0, 160:168 b1, 168 b2 (unchanged)

Compute dtype: KERNEL_DT env = "bf16" (default, full-rate matmuls,
rel err ~1e-2) or "fp32" (native fp32 matmuls, 4 passes, slower).
"""

import os
import numpy as np

B, CIN, H, W = 4, 128, 160, 160
K = 100
CH = 8
OUT = 8
STRIDE = 4
EPS = 1e-5
NCORES = 8

ROWS = H // NCORES          # 20 output rows per core
F = W + 2                   # padded row width 162
HALO = 4
RIN = ROWS + 2 * HALO       # 28 input rows per core
P3 = ROWS * F               # 3240 padded pixels per core
NCHUNK = 486                # mask-head / proj free-dim chunk (3 rows)
CONTRACT = CIN * 9          # 1152
GROUPS = [(g * 16, min(16, K - g * 16)) for g in range((K + 15) // 16)]

_CACHE = {}


def _mode():
    return os.environ.get("KERNEL_DT", "bf16")


def _param_perm():
    """new param index -> original param index (169,)

    w0's h-channel axis c is reordered to match the hbase row layout
    (feats rows 0..8, then -gx, -gy rows 8..10): c' = c-2 for feats,
    c'=8 for rel-x (orig c=0), c'=9 for rel-y (orig c=1).
    """
    perm = np.zeros(169, np.int64)
    corder = [2, 3, 4, 5, 6, 7, 8, 9, 0, 1]
    for cp, c in enumerate(corder):
        for o in range(8):
            perm[cp * 8 + o] = o * 10 + c         # w0
    for o in range(8):
        for o2 in range(8):
            perm[80 + o * 8 + o2] = 80 + o2 * 8 + o   # w1 transposed per-instance
    perm[144:169] = np.arange(144, 169)
    return perm


def _host_prep(inputs):
    """Build the 8 per-core input maps (pure numpy indexing + packing)."""
    import ml_dtypes
    cdt_np = np.float32 if _mode() == "fp32" else ml_dtypes.bfloat16

    cnn_feature = np.asarray(inputs["cnn_feature"], np.float32)
    tower_w = np.asarray(inputs["tower_w"], np.float32)
    bn_gamma = np.asarray(inputs["bn_gamma"], np.float32)
    bn_beta = np.asarray(inputs["bn_beta"], np.float32)
    bn_mean = np.asarray(inputs["bn_mean"], np.float32)
    bn_var = np.asarray(inputs["bn_var"], np.float32)
    proj_w = np.asarray(inputs["proj_w"], np.float32)
    proj_b = np.asarray(inputs["proj_b"], np.float32)
    ctrl_w = np.asarray(inputs["ctrl_w"], np.float32)
    ctrl_b = np.asarray(inputs["ctrl_b"], np.float32)
    detection = np.asarray(inputs["detection"])

    x0 = cnn_feature[0]                                   # [128, 160, 160]

    # tower weights as lhsT per tap: twT[i*9+ky*3+kx] = W[i,:,:,ky,kx].T
    twT = np.ascontiguousarray(
        tower_w.transpose(0, 3, 4, 2, 1).reshape(36, 128, 128)).astype(cdt_np)

    # BN scale/shift
    inv = bn_gamma / np.sqrt(bn_var + EPS)                # [4, 128]
    shift = bn_beta - bn_mean * inv                       # [4, 128]

    # controller weights, column-permuted, +bias row, padded to 1280 contract
    perm = _param_perm()
    cw_flat = ctrl_w.reshape(169, CONTRACT)
    cwT = np.zeros((1280, 169), np.float32)
    cwT[:CONTRACT, :] = cw_flat[perm].T
    cwT[CONTRACT, :] = ctrl_b[perm]

    # patches at detection centers, transposed, +ones row; fused with cwT
    # into one tensor so each 128-contract chunk is a single DMA (the fp32
    # matmul codegen allows only one sync wait on its weight-load slot)
    xs = detection[:, 0].astype(np.int64)
    ys = detection[:, 1].astype(np.int64)
    xpad2 = np.pad(x0, ((0, 0), (1, 1), (1, 1)))
    pcw = np.zeros((1280, K + 169), np.float32)
    for k in range(K):
        pcw[:CONTRACT, k] = xpad2[:, ys[k]:ys[k] + 3, xs[k]:xs[k] + 3].ravel()
    pcw[CONTRACT, :K] = 1.0
    pcw[:, K:] = cwT

    # detection centers replicated 8x along partitions, per 16-instance group:
    # detfan[kl*8+o, g] = 4*x_{16g+kl} (cols 0..7), 4*y (cols 7..14)
    det4 = detection.astype(np.float32) * STRIDE
    detfan = np.zeros((128, 14), np.float32)
    for g, (k0, gsz) in enumerate(GROUPS):
        for kl in range(gsz):
            detfan[kl * 8:kl * 8 + 8, g] = det4[k0 + kl, 0]
            detfan[kl * 8:kl * 8 + 8, 7 + g] = det4[k0 + kl, 1]

    onesbd = np.zeros((128, 16), np.float32)
    for kl in range(16):
        onesbd[kl * 8:kl * 8 + 8, kl] = 1.0

    projT = np.ascontiguousarray(proj_w.T).astype(cdt_np)  # [128, 8]
    projb = proj_b.reshape(8, 1).astype(np.float32)

    # per-core padded input slices
    xpad_rows = np.zeros((128, H + 2 * HALO, F), np.float32)
    xpad_rows[:, HALO:HALO + H, 1:161] = x0
    xpad_rows = xpad_rows.astype(cdt_np)

    shared = dict(twT=twT, pcw=pcw.astype(cdt_np), detfan=detfan,
                  onesbd=onesbd.astype(cdt_np), projT=projT, projb=projb)

    in_maps = []
    for c in range(NCORES):
        xin = np.ascontiguousarray(xpad_rows[:, ROWS * c:ROWS * c + RIN, :])

        # bnv[ch, i, region, 0/1] = inv/shift; zeroed for out-of-image regions
        bnv = np.zeros((128, 4, 3, 2), np.float32)
        for i in range(4):
            bnv[:, i, 1, 0] = inv[i]
            bnv[:, i, 1, 1] = shift[i]
            if c != 0:
                bnv[:, i, 0, 0] = inv[i]
                bnv[:, i, 0, 1] = shift[i]
            if c != NCORES - 1:
                bnv[:, i, 2, 0] = inv[i]
                bnv[:, i, 2, 1] = shift[i]

        grid = np.zeros((2, ROWS, F), np.float32)
        gxrow = -(np.arange(W, dtype=np.float32) * STRIDE + STRIDE // 2)
        gyv = -(np.arange(ROWS * c, ROWS * c + ROWS, dtype=np.float32) * STRIDE
                + STRIDE // 2)
        grid[0, :, 1:161] = gxrow[None, :]
        grid[1, :, 1:161] = gyv[:, None]

        in_maps.append(dict(shared, xin=xin,
                            bnv=bnv.reshape(128, 24),
                            grid=grid.reshape(2, ROWS * F).astype(cdt_np)))
    return in_maps


def _build_program(reps=1):
    from contextlib import ExitStack
    import concourse.tile as tile
    from concourse import bacc, mybir

    f32 = mybir.dt.float32
    cdt = f32 if _mode() == "fp32" else mybir.dt.bfloat16
    Relu = mybir.ActivationFunctionType.Relu
    Ident = mybir.ActivationFunctionType.Identity

    nc = bacc.Bacc("TRN2", target_bir_lowering=False, debug=False,
                   enable_asserts=False, detect_race_conditions=False)

    xin_d = nc.dram_tensor("xin", [128, RIN, F], cdt, kind="ExternalInput")
    twT_d = nc.dram_tensor("twT", [36, 128, 128], cdt, kind="ExternalInput")
    bnv_d = nc.dram_tensor("bnv", [128, 24], f32, kind="ExternalInput")
    grid_d = nc.dram_tensor("grid", [2, P3], cdt, kind="ExternalInput")
    pcw_d = nc.dram_tensor("pcw", [1280, K + 169], cdt, kind="ExternalInput")
    detfan_d = nc.dram_tensor("detfan", [128, 14], f32, kind="ExternalInput")
    onesbd_d = nc.dram_tensor("onesbd", [128, 16], cdt, kind="ExternalInput")
    projT_d = nc.dram_tensor("projT", [128, 8], cdt, kind="ExternalInput")
    projb_d = nc.dram_tensor("projb", [8, 1], f32, kind="ExternalInput")
    out_d = nc.dram_tensor("out", [K, ROWS, W], f32, kind="ExternalOutput")

    with tile.TileContext(nc) as tc, ExitStack() as octx:
      for rep in range(reps):
       with ExitStack() as ctx:
        const = ctx.enter_context(tc.tile_pool(name=f"const{rep}", bufs=1))
        prep = ctx.enter_context(tc.tile_pool(name=f"prep{rep}", bufs=1))

        # ---------- phase A: dynamic params P2[k, j'] = patches @ ctrl ----------
        with tc.tile_pool(name="pm", bufs=1) as pm, \
             tc.tile_pool(name="pm_ps", bufs=1, space="PSUM") as pm_ps:
            pc = []
            for i in range(10):
                t = pm.tile([128, K + 169], cdt, tag=f"pc{i}")
                nc.sync.dma_start(out=t[:], in_=pcw_d[128 * i:128 * (i + 1), :])
                pc.append(t)
            p2p = pm_ps.tile([K, 169], f32)
            for i in range(10):
                nc.tensor.matmul(p2p[:], lhsT=pc[i][:, 0:K], rhs=pc[i][:, K:],
                                 start=(i == 0), stop=(i == 9))
            p2 = prep.tile([K, 169], cdt)
            nc.vector.tensor_copy(p2[:], p2p[:])

        # ---------- phase B: head weight assembly (DMA/DVE, overlaps tower) ----
        detfan_sb = const.tile([128, 14], f32)
        nc.sync.dma_start(out=detfan_sb[:], in_=detfan_d[:])
        onesbd_sb = const.tile([128, 16], cdt)
        nc.sync.dma_start(out=onesbd_sb[:], in_=onesbd_d[:])

        # lhsT0 [10, 800]: l0[c, k*8+o] = P2[k, c*8+o]
        l0 = prep.tile([10, 8 * K], cdt)
        for c in range(10):
            nc.sync.dma_start(
                out=l0[c:c + 1, :].rearrange("p (k o) -> p k o", o=8),
                in_=p2[:, c * 8:(c + 1) * 8])

        # fan-out columns: [kl*8+o, g] layout, one column per instance group
        b0fan = prep.tile([128, 7], f32)
        w0xfan = prep.tile([128, 7], f32)
        w0yfan = prep.tile([128, 7], f32)
        b1fan = prep.tile([128, 7], f32)
        w2fan = prep.tile([128, 7], f32)
        b2fan = prep.tile([16, 7], f32)
        for fan in (b0fan, w0xfan, w0yfan, b1fan, w2fan, b2fan):
            nc.gpsimd.memset(fan[:], 0.0)
        bd1 = []
        for g, (k0, gsz) in enumerate(GROUPS):
            gp = gsz * 8
            bd = prep.tile([gp, gp], cdt, tag=f"bd{g}")
            nc.gpsimd.memset(bd[:], 0.0)
            for kl in range(gsz):
                k = k0 + kl
                nc.sync.dma_start(out=bd[kl * 8:kl * 8 + 8, kl * 8:kl * 8 + 8],
                                  in_=p2[k:k + 1, 80:144])
            bd1.append(bd)
            for fan, c0 in ((b0fan, 152), (w0xfan, 64), (w0yfan, 72),
                            (b1fan, 160), (w2fan, 144)):
                nc.gpsimd.dma_start(out=fan[0:gp, g:g + 1],
                                    in_=p2[k0:k0 + gsz, c0:c0 + 8])
            nc.gpsimd.dma_start(out=b2fan[0:gsz, g:g + 1],
                                in_=p2[k0:k0 + gsz, 168:169])

        # beta0fan = b0fan + w0xfan*4x + w0yfan*4y  (per-instance bias, layer 0)
        beta0fan = prep.tile([128, 7], f32)
        tmpf = prep.tile([128, 7], f32)
        nc.vector.tensor_mul(beta0fan[:], w0xfan[:], detfan_sb[:, 0:7])
        nc.vector.tensor_mul(tmpf[:], w0yfan[:], detfan_sb[:, 7:14])
        nc.vector.tensor_add(beta0fan[:], beta0fan[:], tmpf[:])
        nc.vector.tensor_add(beta0fan[:], beta0fan[:], b0fan[:])

        # ---------- phase C: conv tower on this core's row slice ----------
        tw_sb = []
        for t in range(36):
            wt = const.tile([128, 128], cdt, tag=f"tw{t}")
            nc.sync.dma_start(out=wt[:], in_=twT_d[t])
            tw_sb.append(wt)
        bnv_sb = const.tile([128, 24], f32)
        nc.sync.dma_start(out=bnv_sb[:], in_=bnv_d[:])

        hbase = const.tile([10, P3], cdt)
        nc.sync.dma_start(out=hbase[8:10, :], in_=grid_d[:])

        with tc.tile_pool(name="conv", bufs=1) as convp, \
             tc.tile_pool(name="conv_ps", bufs=4, space="PSUM") as conv_ps:
            xbuf = convp.tile([128, RIN * F + 2], cdt, tag="xbuf")
            nc.gpsimd.memset(xbuf[:, 0:1], 0.0)
            nc.gpsimd.memset(xbuf[:, 1 + RIN * F:], 0.0)
            nc.sync.dma_start(out=xbuf[:, 1:1 + RIN * F], in_=xin_d[:])

            cur = xbuf
            rcur = RIN
            for i in range(4):
                rout = rcur - 2
                obuf = convp.tile([128, rout * F + 2], cdt, tag=f"c{i}")
                nc.gpsimd.memset(obuf[:], 0.0)
                obuf3 = obuf[:, 1:1 + rout * F].rearrange("p (r c) -> p r c", c=F)

                T = 3 - i  # out-of-image candidate rows at top/bottom
                bounds = sorted({0, T, rout - T, rout})
                for r0 in range(0, rout, 3):
                    nr = min(3, rout - r0)
                    ps = conv_ps.tile([128, nr * F], f32, tag="cps")
                    for t, (ky, kx) in enumerate(
                            (ky, kx) for ky in range(3) for kx in range(3)):
                        off = 1 + (r0 + ky) * F + kx - 1
                        nc.tensor.matmul(
                            ps[:], lhsT=tw_sb[i * 9 + t][:],
                            rhs=cur[:, off:off + nr * F],
                            start=(t == 0), stop=(t == 8))
                    ps3 = ps[:].rearrange("p (r c) -> p r c", c=F)
                    # split chunk rows by (top|mid|bot) BN regions
                    for rs, re in zip(bounds[:-1], bounds[1:]):
                        a, b = max(rs, r0), min(re, r0 + nr)
                        if a >= b:
                            continue
                        reg = 0 if b <= T else (2 if a >= rout - T else 1)
                        sidx = (i * 3 + reg) * 2
                        nc.scalar.activation(
                            out=obuf3[:, a:b, 1:161],
                            in_=ps3[:, a - r0:b - r0, 1:161],
                            func=Relu,
                            scale=bnv_sb[:, sidx:sidx + 1],
                            bias=bnv_sb[:, sidx + 1:sidx + 2])
                cur = obuf
                rcur = rout

            # ---------- proj: mask_feats -> hbase rows 0..8 ----------
            projT_sb = const.tile([128, 8], cdt)
            nc.sync.dma_start(out=projT_sb[:], in_=projT_d[:])
            projb_sb = const.tile([8, 1], f32)
            nc.sync.dma_start(out=projb_sb[:], in_=projb_d[:])

            c4 = cur[:, 1:1 + P3]
            with tc.tile_pool(name="proj_ps", bufs=2, space="PSUM") as proj_ps:
                for n0 in range(0, P3, NCHUNK):
                    nn = min(NCHUNK, P3 - n0)
                    pp = proj_ps.tile([8, nn], f32, tag="pps")
                    nc.tensor.matmul(pp[:], lhsT=projT_sb[:],
                                     rhs=c4[:, n0:n0 + nn],
                                     start=True, stop=True)
                    nc.scalar.activation(out=hbase[0:8, n0:n0 + nn], in_=pp[:],
                                         func=Ident, bias=projb_sb[:, 0:1])

        # ---------- phase D: dynamic mask head ----------
        with tc.tile_pool(name="head", bufs=3) as headp, \
             tc.tile_pool(name="outp", bufs=2) as outp, \
             tc.tile_pool(name="head_ps", bufs=2, space="PSUM") as head_ps:
            for g, (k0, gsz) in enumerate(GROUPS):
                gp = gsz * 8
                outg = outp.tile([gsz, P3], f32, tag="outg")
                for n0 in range(0, P3, NCHUNK):
                    nn = min(NCHUNK, P3 - n0)
                    ps0 = hps0.tile([gp, nn], f32, tag="ps0")
                    nc.tensor.matmul(ps0[:],
                                     lhsT=l0[:, 8 * k0:8 * k0 + gp],
                                     rhs=hbase[:, n0:n0 + nn],
                                     start=True, stop=True)
                    h1c = headp.tile([gp, nn], cdt, tag="h1c")
                    nc.scalar.activation(out=h1c[:], in_=ps0[:], func=Relu,
                                         bias=beta0fan[0:gp, g:g + 1])
                    ps1 = hps1.tile([gp, nn], f32, tag="ps1")
                    nc.tensor.matmul(ps1[:], lhsT=bd1[g][:], rhs=h1c[:],
                                     start=True, stop=True)
                    h2c = headp.tile([gp, nn], cdt, tag="h2c")
                    nc.scalar.activation(out=h2c[:], in_=ps1[:], func=Relu,
                                         bias=b1fan[0:gp, g:g + 1])
                    h2w = headp.tile([gp, nn], cdt, tag="h2w")
                    nc.vector.tensor_scalar_mul(h2w[:], h2c[:],
                                                w2fan[0:gp, g:g + 1])
                    ps2 = hps2.tile([gsz, nn], f32, tag="ps2")
                    nc.tensor.matmul(ps2[:],
                                     lhsT=onesbd_sb[0:gp, 0:gsz], rhs=h2w[:],
                                     start=True, stop=True)
                    nc.scalar.activation(out=outg[:, n0:n0 + nn], in_=ps2[:],
                                         func=Ident,
                                         bias=b2fan[0:gsz, g:g + 1])
                outg3 = outg[:].rearrange("p (r c) -> p r c", c=F)
                nc.sync.dma_start(out=out_d[k0:k0 + gsz, 0:10, :],
                                  in_=outg3[:, 0:10, 1:161])
                nc.sync.dma_start(out=out_d[k0:k0 + gsz, 10:20, :],
                                  in_=outg3[:, 10:20, 1:161])
    nc.compile()
    return nc


def _get_program(reps=1):
    key = (_mode(), reps)
    if key not in _CACHE:
        _CACHE[key] = _build_program(reps)
    return _CACHE[key]


def _run(in_maps, trace=False, reps=1, **kwargs):
    from concourse.bass_utils import run_bass_kernel_spmd
    nc = _get_program(reps)
    return run_bass_kernel_spmd(nc, in_maps, core_ids=list(range(NCORES)),
                                trace=trace, **kwargs)


def kernel(**inputs) -> np.ndarray:
    in_maps = _host_prep(inputs)
    res = _run(in_maps)
    out = np.concatenate([res.results[c]["out"] for c in range(NCORES)], axis=1)
    return out.astype(np.float32)
